# revision 11
# baseline (speedup 1.0000x reference)
"""DopDense forward: relu(x @ (w * mult) + b) on 8 trn2 NeuronCores.

Key algebra: w_new = w * mult (per-column scaling) commutes with the matmul,
so out = relu((x @ w) * mult[None, :] + b).  We compute y^T tiles (units on
partitions, batch on free axis) so the per-column mult/bias become
per-partition scale/bias of a fused Relu eviction (scalar-engine activation
or a 2-op vector tensor_scalar).

Sharding: data-parallel over the batch axis (8192 rows/core); w, dop state
replicated.  mult is computed on-device from w, dop_weights_old, indicator,
batch_ctr; the big matmul runs in bf16, everything else fp32.

The kernel is memory-bound (~26 MB/core), so DMA traffic is spread across
the sync HWDGE, scalar HWDGE and gpsimd SWDGE queues, with few large DMAs
(each DMA issue costs ~650ns of engine time).
"""

import numpy as np
import ml_dtypes

import concourse.bass as bass
import concourse.mybir as mybir
import concourse.tile as tile
from concourse import bacc
from concourse.bass_utils import run_bass_kernel_spmd

F32 = mybir.dt.float32
BF16 = mybir.dt.bfloat16
AF = mybir.ActivationFunctionType
ALU = mybir.AluOpType
BF16_NP = np.dtype(ml_dtypes.bfloat16)

N_CORES = 8
B = 65536
NIN = 512
UNITS = 512
N_DOP = 128
SHARD = B // N_CORES          # 8192 batch rows per core
W = 1024                      # batch window per psum tile (2 PSUM banks)
NWP = SHARD // W              # 8 windows per core
KC = NIN // 128               # 4 contraction chunks
CC = UNITS // 128             # 4 unit chunks
THRESHOLD = 0.0
REF_PERIOD = 2.0

# Static dopaminergic-column index math (mirrors reference.py exactly)
DOP_IDX = np.linspace(1, UNITS - 1, N_DOP, dtype=np.int32)
LEFT_OK = ~np.isin(DOP_IDX - 1, DOP_IDX)
RIGHT_OK = ~np.isin(DOP_IDX + 1, DOP_IDX)
LCOL = (DOP_IDX - 1) % UNITS
RCOL = (DOP_IDX + 1) % UNITS


def _static_masks():
    # scatter matrices, packed side by side: [j, cc*128+m] for L, then R
    lr = np.zeros((128, 2 * UNITS), np.float32)
    for j in range(N_DOP):
        lr[j, LCOL[j]] = 1.0
        lr[j, UNITS + RCOL[j]] = 1.0
    return lr


LRMAT = _static_masks()
LOK10 = LEFT_OK.astype(np.float32) * np.float32(10.0 / NIN)
ROK10 = RIGHT_OK.astype(np.float32) * np.float32(10.0 / NIN)

_CACHED_NC = None


def build_nc():
    global _CACHED_NC
    if _CACHED_NC is not None:
        return _CACHED_NC
    nc = bacc.Bacc("TRN2", target_bir_lowering=False, debug=False,
                   num_swdge_queues=2)

    xt = nc.dram_tensor("xt", [NWP, 128, KC * W], BF16, kind="ExternalInput")
    # w chunks packed as [128, (k*CC+c)*128 + m] (bf16, matmul stationary)
    wkb = nc.dram_tensor("wkb", [128, KC * CC * 128], BF16, kind="ExternalInput")
    # dop columns of w^T / old^T side by side: [128, 0:512]=w, [128, 512:1024]=old
    wod = nc.dram_tensor("wod", [128, 2 * NIN], F32, kind="ExternalInput")
    lrmat = nc.dram_tensor("lrmat", [128, 2 * UNITS], F32, kind="ExternalInput")
    # 8 per-partition vectors: lok10, rok10, indicator, batch_ctr, b0..b3
    vecs = nc.dram_tensor("vecs", [128, 8], F32, kind="ExternalInput")
    yt = nc.dram_tensor("yt", [NWP, 128, CC * W], F32, kind="ExternalOutput")

    with tile.TileContext(nc) as tc:
        with (
            tc.tile_pool(name="const", bufs=1) as const,
            tc.tile_pool(name="aux", bufs=1) as aux,
            tc.tile_pool(name="xa", bufs=4) as xpool,
            tc.tile_pool(name="ob", bufs=4) as opool,
            tc.tile_pool(name="tmp", bufs=2) as tpool,
        ):
            # ---------- input DMAs: few, large, spread over 3 queues ----------
            # aux-critical inputs lead the sync queue (they gate the mult
            # computation, which gates every eviction)
            v_sb = const.tile([128, 8], F32, tag="v")
            nc.sync.dma_start(v_sb[:], vecs[:])
            wod_sb = aux.tile([128, 2 * NIN], F32, tag="wod")
            nc.sync.dma_start(wod_sb[:], wod[:])
            lr_sb = const.tile([128, 2 * UNITS], F32, tag="lr")
            nc.sync.dma_start(lr_sb[:], lrmat[:])
            wk_sb = const.tile([128, KC * CC * 128], BF16, tag="wk")
            nc.scalar.dma_start(wk_sb[:], wkb[:])

            def wk_tile(k, c):
                i = k * CC + c
                return wk_sb[:, i * 128:(i + 1) * 128]

            # x windows: wp0 split for fast start; then alternate sync/gpsimd.
            # Rolling prefetch (depth 3) so queue order matches readiness.
            xa_tiles = {}

            def load_xa(wp):
                xa = xpool.tile([128, KC * W], BF16, tag="xa")
                if wp == 0:
                    nc.sync.dma_start(xa[:, :2 * W], xt[0][:, :2 * W])
                    nc.scalar.dma_start(xa[:, 2 * W:], xt[0][:, 2 * W:])
                elif wp % 2 == 1:
                    nc.gpsimd.dma_start(xa[:], xt[wp])
                else:
                    nc.sync.dma_start(xa[:], xt[wp])
                xa_tiles[wp] = xa

            for wp in range(3):
                load_xa(wp)

            # ---------- aux compute: dd[j] = sum_i |w[i,d_j] - old[i,d_j]| ----
            dch = aux.tile([128, NIN], F32, tag="dch")
            nc.vector.tensor_tensor(dch[:], wod_sb[:, :NIN], wod_sb[:, NIN:],
                                    op=ALU.subtract)
            dd = const.tile([128, 1], F32, tag="dd")
            nc.vector.tensor_reduce(
                dd[:], dch[:], axis=mybir.AxisListType.X, op=ALU.add,
                apply_absolute_value=True,
            )
            # active = (dd > THRESHOLD) & ((batch_ctr - indicator) > REF_PERIOD)
            t1 = const.tile([128, 1], F32, tag="t1")
            nc.vector.tensor_tensor(t1[:], v_sb[:, 3:4], v_sb[:, 2:3],
                                    op=ALU.subtract)
            c2 = const.tile([128, 1], F32, tag="c2")
            nc.vector.tensor_scalar(c2[:], t1[:], REF_PERIOD, None, op0=ALU.is_gt)
            c1 = const.tile([128, 1], F32, tag="c1")
            nc.vector.tensor_scalar(c1[:], dd[:], THRESHOLD, None, op0=ALU.is_gt)
            av = const.tile([128, 1], F32, tag="av")
            nc.vector.tensor_tensor(av[:], c1[:], c2[:], op=ALU.mult)
            da = const.tile([128, 1], F32, tag="da")
            nc.vector.tensor_tensor(da[:], dd[:], av[:], op=ALU.mult)
            lf1 = const.tile([128, 1], F32, tag="lf1")
            nc.vector.tensor_tensor(lf1[:], da[:], v_sb[:, 0:1], op=ALU.mult)
            rf1 = const.tile([128, 1], F32, tag="rf1")
            nc.vector.tensor_tensor(rf1[:], da[:], v_sb[:, 1:2], op=ALU.mult)
            llf = const.tile([128, 1], F32, tag="llf")
            nc.scalar.activation(llf[:], lf1[:], AF.Ln, bias=1.0)
            lrf = const.tile([128, 1], F32, tag="lrf")
            nc.scalar.activation(lrf[:], rf1[:], AF.Ln, bias=1.0)

            # multiplicative scatter to columns via log-space accumulate
            mult_sb = []
            with tc.tile_pool(name="psx", bufs=1, space="PSUM") as psaux:
                for cc in range(CC):
                    ml_ps = psaux.tile([128, 1], F32, tag="auxps")
                    nc.tensor.matmul(ml_ps[:], lr_sb[:, cc * 128:(cc + 1) * 128],
                                     llf[:], start=True, stop=False)
                    nc.tensor.matmul(ml_ps[:],
                                     lr_sb[:, UNITS + cc * 128:UNITS + (cc + 1) * 128],
                                     lrf[:], start=False, stop=True)
                    m = const.tile([128, 1], F32, tag=f"mult{cc}")
                    nc.scalar.activation(m[:], ml_ps[:], AF.Exp)
                    mult_sb.append(m)

            # ---------- main: y^T = (w^T x^T) scaled+biased+relu ----------
            # Window pairs share each stationary weight across 4 matmuls,
            # reducing LDWEIGHTS pressure on the PE.
            def evict(ps, ob, c):
                obs = ob[:, c * W:(c + 1) * W]
                if c < 3:
                    nc.scalar.activation(
                        obs, ps[:], AF.Relu,
                        bias=v_sb[:, 4 + c:5 + c], scale=mult_sb[c][:])
                else:
                    tmp = tpool.tile([128, W], F32, tag="evt")
                    nc.vector.tensor_scalar(
                        tmp[:], ps[:], mult_sb[c][:], v_sb[:, 4 + c:5 + c],
                        op0=ALU.mult, op1=ALU.add)
                    nc.vector.tensor_scalar(
                        obs, tmp[:], 0.0, None, op0=ALU.max)

            with tc.tile_pool(name="ps", bufs=4, space="PSUM") as pspool:
                for sw in range(NWP // 2):
                    if sw + 1 < NWP // 2:
                        load_xa(2 * (sw + 1))
                        load_xa(2 * (sw + 1) + 1)
                    wpa, wpb = 2 * sw, 2 * sw + 1
                    xaa, xab = xa_tiles[wpa], xa_tiles[wpb]
                    oba = opool.tile([128, CC * W], F32, tag="ob")
                    obb = opool.tile([128, CC * W], F32, tag="ob")
                    for c in range(CC):
                        psa = pspool.tile([128, W], F32, tag="mps")
                        psb = pspool.tile([128, W], F32, tag="mps")
                        for k in range(KC):
                            for ps, xa in ((psa, xaa), (psb, xab)):
                                for s in range(W // 512):
                                    nc.tensor.matmul(
                                        ps[:, s * 512:(s + 1) * 512],
                                        wk_tile(k, c),
                                        xa[:, k * W + s * 512: k * W + (s + 1) * 512],
                                        start=(k == 0), stop=(k == KC - 1),
                                    )
                        evict(psa, oba, c)
                        evict(psb, obb, c)
                        # drain output halves as soon as each pair is evicted
                        if c == 1:
                            nc.scalar.dma_start(yt[wpa][:, :2 * W], oba[:, :2 * W])
                            nc.scalar.dma_start(yt[wpb][:, :2 * W], obb[:, :2 * W])
                        elif c == 3:
                            eng = nc.sync if sw % 2 == 0 else nc.gpsimd
                            eng.dma_start(yt[wpa][:, 2 * W:], oba[:, 2 * W:])
                            eng.dma_start(yt[wpb][:, 2 * W:], obb[:, 2 * W:])

    nc.compile()
    _CACHED_NC = nc
    return nc


LAST_RESULTS = None


def kernel(x, w, b, dop_weights_old, indicator, batch_ctr):
    global LAST_RESULTS
    x = np.asarray(x, dtype=np.float32)
    w = np.ascontiguousarray(np.asarray(w, dtype=np.float32))
    b_arr = np.asarray(b, dtype=np.float32)
    old = np.asarray(dop_weights_old, dtype=np.float32)
    ind = np.asarray(indicator, dtype=np.float32)
    bc_val = float(np.asarray(batch_ctr).item())

    nc = build_nc()

    # replicated (per-core identical) inputs; all reshapes/gathers are pure
    # data marshaling -- every arithmetic op happens on device
    wkb = np.ascontiguousarray(
        w.reshape(KC, 128, CC, 128).transpose(1, 0, 2, 3)
    ).reshape(128, KC * CC * 128).astype(BF16_NP)
    wod = np.concatenate([w.T[DOP_IDX], old.T[DOP_IDX]], axis=1)
    wod = np.ascontiguousarray(wod)
    vecs = np.stack(
        [LOK10, ROK10, ind.astype(np.float32),
         np.full(128, bc_val, np.float32)]
        + [b_arr[c * 128:(c + 1) * 128] for c in range(CC)], axis=1)
    vecs = np.ascontiguousarray(vecs.astype(np.float32))

    common = dict(wkb=wkb, wod=wod, lrmat=LRMAT, vecs=vecs)

    xbf = x.astype(BF16_NP)
    in_maps = []
    for i in range(N_CORES):
        xs = xbf[i * SHARD:(i + 1) * SHARD]          # [8192, 512]
        xtc = np.ascontiguousarray(
            xs.reshape(NWP, W, KC, 128).transpose(0, 3, 2, 1)
        ).reshape(NWP, 128, KC * W)
        in_maps.append(dict(common, xt=xtc))

    res = run_bass_kernel_spmd(nc, in_maps, core_ids=list(range(N_CORES)))
    LAST_RESULTS = res

    out = np.empty((B, UNITS), np.float32)
    for i in range(N_CORES):
        ytc = res.results[i]["yt"].reshape(NWP, 128, CC, W)
        out[i * SHARD:(i + 1) * SHARD] = (
            ytc.transpose(0, 3, 2, 1).reshape(SHARD, UNITS))
    return out


# revision 12
# speedup vs baseline: 1.1257x; 1.1257x over previous
"""DopDense forward: relu(x @ (w * mult) + b) on 8 trn2 NeuronCores.

Key algebra: w_new = w * mult (per-column scaling) commutes with the matmul,
so out = relu((x @ w) * mult[None, :] + b).  We compute y^T tiles (units on
partitions, batch on free axis) so the per-column mult/bias become
per-partition scale/bias of a fused Relu eviction (scalar-engine activation
or a 2-op vector tensor_scalar).

mult is computed on device: dd[j] = sum_i |w[i,d_j] - old[i,d_j]| (vector
engine), gating logic in j-space, then a multiplicative scatter to columns
as mult = (1 + L^T lfm1) * (1 + R^T rfm1) -- left/right target columns are
each unique, and the single collision (column 0) is handled exactly by the
product.  L/R are built on device from an iota constant via is_equal.

Sharding: data-parallel over the batch axis (8192 rows/core); w, dop state
replicated.  The big matmul runs in bf16, everything else fp32.  The kernel
is memory-bound (~25 MB/core), so DMA traffic is spread across the sync
HWDGE, scalar HWDGE and gpsimd SWDGE queues with few large DMAs.
"""

import numpy as np
import ml_dtypes

import concourse.bass as bass
import concourse.mybir as mybir
import concourse.tile as tile
from concourse import bacc
from concourse.bass_utils import run_bass_kernel_spmd

F32 = mybir.dt.float32
BF16 = mybir.dt.bfloat16
AF = mybir.ActivationFunctionType
ALU = mybir.AluOpType
BF16_NP = np.dtype(ml_dtypes.bfloat16)

N_CORES = 8
B = 65536
NIN = 512
UNITS = 512
N_DOP = 128
SHARD = B // N_CORES          # 8192 batch rows per core
W = 1024                      # batch window per psum tile (2 PSUM banks)
NWP = SHARD // W              # 8 windows per core
KC = NIN // 128               # 4 contraction chunks
CC = UNITS // 128             # 4 unit chunks
THRESHOLD = 0.0
REF_PERIOD = 2.0

# Static dopaminergic-column index math (mirrors reference.py exactly)
DOP_IDX = np.linspace(1, UNITS - 1, N_DOP, dtype=np.int32)
LEFT_OK = ~np.isin(DOP_IDX - 1, DOP_IDX)
RIGHT_OK = ~np.isin(DOP_IDX + 1, DOP_IDX)
LCOL = (DOP_IDX - 1) % UNITS
RCOL = (DOP_IDX + 1) % UNITS

LOK10 = LEFT_OK.astype(np.float32) * np.float32(10.0 / NIN)
ROK10 = RIGHT_OK.astype(np.float32) * np.float32(10.0 / NIN)

_CACHED_NC = None


def build_nc():
    global _CACHED_NC
    if _CACHED_NC is not None:
        return _CACHED_NC
    nc = bacc.Bacc("TRN2", target_bir_lowering=False, debug=False,
                   num_swdge_queues=2)

    xt = nc.dram_tensor("xt", [NWP, 128, KC * W], BF16, kind="ExternalInput")
    # w chunks packed as [128, (k*CC+c)*128 + m] (bf16, matmul stationary)
    wkb = nc.dram_tensor("wkb", [128, KC * CC * 128], BF16, kind="ExternalInput")
    # dop columns of w^T and old^T (separate tensors -> two DMA queues)
    wdt = nc.dram_tensor("wdt", [128, NIN], F32, kind="ExternalInput")
    odt = nc.dram_tensor("odt", [128, NIN], F32, kind="ExternalInput")
    # iota pattern: every row = 0..127 (for building scatter masks on device)
    iot = nc.dram_tensor("iot", [128, 128], F32, kind="ExternalInput")
    # 16 per-partition vectors: lok10, rok10, indicator, batch_ctr, b0..b3,
    # lcol-cc*128 (cc=0..3), rcol-cc*128 (cc=0..3)
    vecs = nc.dram_tensor("vecs", [128, 16], F32, kind="ExternalInput")
    yt = nc.dram_tensor("yt", [NWP, 128, CC * W], F32, kind="ExternalOutput")

    with tile.TileContext(nc) as tc:
        with (
            tc.tile_pool(name="const", bufs=1) as const,
            tc.tile_pool(name="aux", bufs=1) as aux,
            tc.tile_pool(name="xa", bufs=4) as xpool,
            tc.tile_pool(name="ob", bufs=4) as opool,
            tc.tile_pool(name="tmp", bufs=2) as tpool,
        ):
            # ---------- input DMAs: few, large, spread over 3 queues ----------
            # aux-critical inputs lead their queues (they gate mult, which
            # gates every eviction)
            v_sb = const.tile([128, 16], F32, tag="v")
            nc.sync.dma_start(v_sb[:], vecs[:])
            io_sb = const.tile([128, 128], F32, tag="io")
            nc.sync.dma_start(io_sb[:], iot[:])
            wd_sb = aux.tile([128, NIN], F32, tag="wd")
            nc.sync.dma_start(wd_sb[:], wdt[:])
            wk_sb = const.tile([128, KC * CC * 128], BF16, tag="wk")
            nc.scalar.dma_start(wk_sb[:], wkb[:])
            od_sb = aux.tile([128, NIN], F32, tag="od")
            nc.scalar.dma_start(od_sb[:], odt[:])

            def wk_tile(k, c):
                i = k * CC + c
                return wk_sb[:, i * 128:(i + 1) * 128]

            # x windows: wp0 split for fast start; then alternate sync/gpsimd.
            xa_tiles = {}

            def load_xa(wp):
                xa = xpool.tile([128, KC * W], BF16, tag="xa")
                if wp == 0:
                    nc.sync.dma_start(xa[:, :2 * W], xt[0][:, :2 * W])
                    nc.scalar.dma_start(xa[:, 2 * W:], xt[0][:, 2 * W:])
                elif wp % 2 == 1:
                    nc.gpsimd.dma_start(xa[:], xt[wp])
                else:
                    nc.sync.dma_start(xa[:], xt[wp])
                xa_tiles[wp] = xa

            for wp in range(3):
                load_xa(wp)

            # scatter masks from iota while waiting on wd/od:
            # L_cc[j, m] = 1 iff LCOL[j] == cc*128 + m
            lmask, rmask = [], []
            for cc in range(CC):
                lt = const.tile([128, 128], F32, tag=f"lm{cc}")
                nc.vector.tensor_scalar(lt[:], io_sb[:], v_sb[:, 8 + cc:9 + cc],
                                        None, op0=ALU.is_equal)
                lmask.append(lt)
                rt = const.tile([128, 128], F32, tag=f"rm{cc}")
                nc.vector.tensor_scalar(rt[:], io_sb[:], v_sb[:, 12 + cc:13 + cc],
                                        None, op0=ALU.is_equal)
                rmask.append(rt)

            # ---------- aux compute: dd[j] = sum_i |w[i,d_j] - old[i,d_j]| ----
            dch = aux.tile([128, NIN], F32, tag="dch")
            nc.vector.tensor_tensor(dch[:], wd_sb[:], od_sb[:], op=ALU.subtract)
            dd = const.tile([128, 1], F32, tag="dd")
            nc.vector.tensor_reduce(
                dd[:], dch[:], axis=mybir.AxisListType.X, op=ALU.add,
                apply_absolute_value=True,
            )
            # active = (dd > THRESHOLD) & ((batch_ctr - indicator) > REF_PERIOD)
            t1 = const.tile([128, 1], F32, tag="t1")
            nc.vector.tensor_tensor(t1[:], v_sb[:, 3:4], v_sb[:, 2:3],
                                    op=ALU.subtract)
            c2 = const.tile([128, 1], F32, tag="c2")
            nc.vector.tensor_scalar(c2[:], t1[:], REF_PERIOD, None, op0=ALU.is_gt)
            c1 = const.tile([128, 1], F32, tag="c1")
            nc.vector.tensor_scalar(c1[:], dd[:], THRESHOLD, None, op0=ALU.is_gt)
            av = const.tile([128, 1], F32, tag="av")
            nc.vector.tensor_tensor(av[:], c1[:], c2[:], op=ALU.mult)
            da = const.tile([128, 1], F32, tag="da")
            nc.vector.tensor_tensor(da[:], dd[:], av[:], op=ALU.mult)
            lf1 = const.tile([128, 1], F32, tag="lf1")
            nc.vector.tensor_tensor(lf1[:], da[:], v_sb[:, 0:1], op=ALU.mult)
            rf1 = const.tile([128, 1], F32, tag="rf1")
            nc.vector.tensor_tensor(rf1[:], da[:], v_sb[:, 1:2], op=ALU.mult)

            # additive scatters, then mult = (1 + L^T lfm1) * (1 + R^T rfm1)
            mult_sb = []
            with tc.tile_pool(name="psx", bufs=2, space="PSUM") as psaux:
                for cc in range(CC):
                    psl = psaux.tile([128, 1], F32, tag="auxps")
                    nc.tensor.matmul(psl[:], lmask[cc][:], lf1[:],
                                     start=True, stop=True)
                    psr = psaux.tile([128, 1], F32, tag="auxps")
                    nc.tensor.matmul(psr[:], rmask[cc][:], rf1[:],
                                     start=True, stop=True)
                    ls1 = const.tile([128, 1], F32, tag=f"ls{cc}")
                    nc.vector.tensor_scalar(ls1[:], psl[:], 1.0, None, op0=ALU.add)
                    rs1 = const.tile([128, 1], F32, tag=f"rs{cc}")
                    nc.vector.tensor_scalar(rs1[:], psr[:], 1.0, None, op0=ALU.add)
                    m = const.tile([128, 1], F32, tag=f"mult{cc}")
                    nc.vector.tensor_tensor(m[:], ls1[:], rs1[:], op=ALU.mult)
                    mult_sb.append(m)

            # ---------- main: y^T = (w^T x^T) scaled+biased+relu ----------
            # Window pairs share each stationary weight across 4 matmuls.
            def evict_act(ps, ob, c):
                nc.scalar.activation(
                    ob[:, c * W:(c + 1) * W], ps[:], AF.Relu,
                    bias=v_sb[:, 4 + c:5 + c], scale=mult_sb[c][:])

            def evict_dve(ps, ob, c):
                tmp = tpool.tile([128, W], F32, tag="evt")
                nc.vector.tensor_scalar(
                    tmp[:], ps[:], mult_sb[c][:], v_sb[:, 4 + c:5 + c],
                    op0=ALU.mult, op1=ALU.add)
                nc.vector.tensor_scalar(
                    ob[:, c * W:(c + 1) * W], tmp[:], 0.0, None, op0=ALU.max)

            NSW = NWP // 2
            with tc.tile_pool(name="ps", bufs=4, space="PSUM") as pspool:
                for sw in range(NSW):
                    if sw + 1 < NSW:
                        load_xa(2 * (sw + 1))
                        load_xa(2 * (sw + 1) + 1)
                    wpa, wpb = 2 * sw, 2 * sw + 1
                    xaa, xab = xa_tiles[wpa], xa_tiles[wpb]
                    oba = opool.tile([128, CC * W], F32, tag="ob")
                    obb = opool.tile([128, CC * W], F32, tag="ob")
                    for c in range(CC):
                        psa = pspool.tile([128, W], F32, tag="mps")
                        psb = pspool.tile([128, W], F32, tag="mps")
                        for k in range(KC):
                            for ps, xa in ((psa, xaa), (psb, xab)):
                                for s in range(W // 512):
                                    nc.tensor.matmul(
                                        ps[:, s * 512:(s + 1) * 512],
                                        wk_tile(k, c),
                                        xa[:, k * W + s * 512: k * W + (s + 1) * 512],
                                        start=(k == 0), stop=(k == KC - 1),
                                    )
                        # 5 ACT + 3 DVE evictions per superwindow
                        if c < 2:
                            evict_act(psa, oba, c)
                            evict_act(psb, obb, c)
                        elif c == 2:
                            evict_act(psa, oba, c)
                            evict_dve(psb, obb, c)
                        else:
                            evict_dve(psa, oba, c)
                            evict_dve(psb, obb, c)
                        # drain output halves as soon as each pair is evicted
                        last = sw == NSW - 1
                        if c == 1:
                            nc.scalar.dma_start(yt[wpa][:, :2 * W], oba[:, :2 * W])
                            nc.scalar.dma_start(yt[wpb][:, :2 * W], obb[:, :2 * W])
                        elif c == 2 and last:
                            nc.sync.dma_start(yt[wpa][:, 2 * W:3 * W],
                                              oba[:, 2 * W:3 * W])
                            nc.gpsimd.dma_start(yt[wpb][:, 2 * W:3 * W],
                                                obb[:, 2 * W:3 * W])
                        elif c == 3:
                            if last:
                                nc.sync.dma_start(yt[wpa][:, 3 * W:],
                                                  oba[:, 3 * W:])
                                nc.gpsimd.dma_start(yt[wpb][:, 3 * W:],
                                                    obb[:, 3 * W:])
                            else:
                                eng = nc.sync if sw % 2 == 0 else nc.gpsimd
                                eng.dma_start(yt[wpa][:, 2 * W:], oba[:, 2 * W:])
                                eng.dma_start(yt[wpb][:, 2 * W:], obb[:, 2 * W:])

    nc.compile()
    _CACHED_NC = nc
    return nc


LAST_RESULTS = None


def kernel(x, w, b, dop_weights_old, indicator, batch_ctr):
    global LAST_RESULTS
    x = np.asarray(x, dtype=np.float32)
    w = np.ascontiguousarray(np.asarray(w, dtype=np.float32))
    b_arr = np.asarray(b, dtype=np.float32)
    old = np.asarray(dop_weights_old, dtype=np.float32)
    ind = np.asarray(indicator, dtype=np.float32)
    bc_val = float(np.asarray(batch_ctr).item())

    nc = build_nc()

    # replicated (per-core identical) inputs; all reshapes/gathers are pure
    # data marshaling -- every arithmetic op happens on device
    wkb = np.ascontiguousarray(
        w.reshape(KC, 128, CC, 128).transpose(1, 0, 2, 3)
    ).reshape(128, KC * CC * 128).astype(BF16_NP)
    wdt = np.ascontiguousarray(w.T[DOP_IDX])
    odt = np.ascontiguousarray(old.T[DOP_IDX])
    iot = np.broadcast_to(np.arange(128, dtype=np.float32), (128, 128))
    iot = np.ascontiguousarray(iot)
    vcols = [LOK10, ROK10, ind.astype(np.float32),
             np.full(128, bc_val, np.float32)]
    vcols += [b_arr[c * 128:(c + 1) * 128] for c in range(CC)]
    vcols += [(LCOL - cc * 128).astype(np.float32) for cc in range(CC)]
    vcols += [(RCOL - cc * 128).astype(np.float32) for cc in range(CC)]
    vecs = np.ascontiguousarray(np.stack(vcols, axis=1).astype(np.float32))

    common = dict(wkb=wkb, wdt=wdt, odt=odt, iot=iot, vecs=vecs)

    xbf = x.astype(BF16_NP)
    in_maps = []
    for i in range(N_CORES):
        xs = xbf[i * SHARD:(i + 1) * SHARD]          # [8192, 512]
        xtc = np.ascontiguousarray(
            xs.reshape(NWP, W, KC, 128).transpose(0, 3, 2, 1)
        ).reshape(NWP, 128, KC * W)
        in_maps.append(dict(common, xt=xtc))

    res = run_bass_kernel_spmd(nc, in_maps, core_ids=list(range(N_CORES)))
    LAST_RESULTS = res

    out = np.empty((B, UNITS), np.float32)
    for i in range(N_CORES):
        ytc = res.results[i]["yt"].reshape(NWP, 128, CC, W)
        out[i * SHARD:(i + 1) * SHARD] = (
            ytc.transpose(0, 3, 2, 1).reshape(SHARD, UNITS))
    return out
